# revision 2
# baseline (speedup 1.0000x reference)
"""CPhaseLayer kernel for Trainium2 (8 NeuronCores, SPMD data-parallel).

The reference computes out = einsum('bcn,nm->bcm', x, tmat) with
x [4096, 2, 8192] f32 and tmat [8192, 8192] f32 where tmat is a Kronecker
product of CPHASE = diag(1,1,-1,1) and I2 gates.  Every factor is diagonal,
so tmat is diagonal with +-1 entries and the matmul reduces EXACTLY to
out[b,c,m] = x[b,c,m] * diag(tmat)[m]  (adding 8191 exact zeros in f32 is
exact, so this is bitwise identical to the dense product).

The device kernel is therefore an elementwise multiply of each row block by
a broadcast sign vector.  Sharding: batch dim split 8 ways (512 batches ->
1024 rows of 8192 per core).  Per core traffic: 32 MiB in + 32 MiB out,
HBM-bound at ~358 GB/s -> ~190 us floor.

The diagonal is extracted from the *runtime* tmat input (not hardcoded), and
diagonality is verified on the host; a host fallback handles the (never
occurring) non-diagonal case.
"""

import numpy as np

B, C, N = 4096, 2, 8192
N_CORES = 8
ROWS = B * C  # 8192 rows of length N
ROWS_PER_CORE = ROWS // N_CORES  # 1024
P = 128  # SBUF partitions

_CACHE = {}


def _build_nc(n_row_tiles: int, free_cols: int):
    """Bass program for one core: out[r, :] = xs[r, :] * d[:]  (d broadcast).

    xs: [ROWS_PER_CORE, N] f32, dr: [P, N] f32 (sign vector replicated on
    all 128 partitions), out: [ROWS_PER_CORE, N] f32.
    """
    import concourse.mybir as mybir
    import concourse.tile as tile
    from concourse import bacc

    f32 = mybir.dt.float32
    nc = bacc.Bacc("TRN2", target_bir_lowering=False, debug=False)

    xs = nc.dram_tensor("xs", [ROWS_PER_CORE, N], f32, kind="ExternalInput")
    dr = nc.dram_tensor("dr", [P, N], f32, kind="ExternalInput")
    out = nc.dram_tensor("out", [ROWS_PER_CORE, N], f32, kind="ExternalOutput")

    n_col_tiles = N // free_cols
    assert ROWS_PER_CORE == n_row_tiles * P

    with tile.TileContext(nc) as tc:
        with (
            tc.tile_pool(name="dpool", bufs=1) as dpool,
            tc.tile_pool(name="xpool", bufs=3) as xpool,
        ):
            dt_ = dpool.tile([P, N], f32)
            nc.sync.dma_start(dt_[:], dr[:, :])
            for i in range(n_row_tiles):
                r0 = i * P
                for j in range(n_col_tiles):
                    c0 = j * free_cols
                    xt = xpool.tile([P, free_cols], f32)
                    nc.sync.dma_start(xt[:], xs[r0 : r0 + P, c0 : c0 + free_cols])
                    nc.vector.tensor_mul(xt[:], xt[:], dt_[:, c0 : c0 + free_cols])
                    nc.sync.dma_start(out[r0 : r0 + P, c0 : c0 + free_cols], xt[:])
    nc.finalize()
    return nc


def _get_nc():
    if "nc" not in _CACHE:
        _CACHE["nc"] = _build_nc(n_row_tiles=ROWS_PER_CORE // P, free_cols=N)
    return _CACHE["nc"]


def _run_device(xs_flat: np.ndarray, d: np.ndarray):
    """xs_flat [8192, 8192] f32  ->  xs_flat * d[None, :] computed on 8 cores."""
    from concourse.bass_utils import run_bass_kernel_spmd

    drep = np.ascontiguousarray(np.broadcast_to(d[None, :], (P, N)))
    in_maps = [
        {
            "xs": np.ascontiguousarray(
                xs_flat[c * ROWS_PER_CORE : (c + 1) * ROWS_PER_CORE]
            ),
            "dr": drep,
        }
        for c in range(N_CORES)
    ]
    res = run_bass_kernel_spmd(_get_nc(), in_maps, core_ids=list(range(N_CORES)))
    out = np.concatenate([r["out"] for r in res.results], axis=0)
    return out


def kernel(x: np.ndarray, tmat: np.ndarray) -> np.ndarray:
    x = np.asarray(x, dtype=np.float32)
    tmat = np.asarray(tmat, dtype=np.float32)
    assert x.shape == (B, C, N) and tmat.shape == (N, N)

    d = np.ascontiguousarray(np.diagonal(tmat))
    if not np.array_equal(tmat, np.diag(d)):
        # Non-diagonal transfer matrix: never happens for CPhaseLayer, but
        # keep a correct host fallback.
        return (x.reshape(ROWS, N).astype(np.float32) @ tmat).reshape(B, C, N)

    xs_flat = np.ascontiguousarray(x).reshape(ROWS, N)
    out = _run_device(xs_flat, d)
    return out.reshape(B, C, N).astype(np.float32)


# revision 3
# speedup vs baseline: 101349.9747x; 101349.9747x over previous
"""CPhaseLayer kernel for Trainium2 (8 NeuronCores, SPMD data-parallel).

The reference computes out = einsum('bcn,nm->bcm', x, tmat) with
x [4096, 2, 8192] f32 and tmat [8192, 8192] f32 where tmat is a Kronecker
product of CPHASE = diag(1,1,-1,1) and I2 gates.  Every factor is diagonal,
so tmat is diagonal with +-1 entries and the matmul reduces EXACTLY to
out[b,c,m] = x[b,c,m] * diag(tmat)[m]  (the other 8191 terms of the f32
dot product are exact zeros, so this is bitwise identical).

Device kernel: elementwise multiply of each 128-row block by the sign
vector (replicated across partitions).  Sharding: batch split 8 ways ->
1024 rows x 8192 per core.  Per-core traffic 64 MiB -> HBM-bound,
~190 us floor at ~358 GB/s.

The diagonal is extracted from the *runtime* tmat input; diagonality is
verified on the host with a fallback for the (never occurring)
non-diagonal case.
"""

import numpy as np

B, C, N = 4096, 2, 8192
N_CORES = 8
ROWS = B * C  # 8192 rows of length N
ROWS_PER_CORE = ROWS // N_CORES  # 1024
P = 128  # SBUF partitions

_CACHE = {}


def _build_nc(free_cols: int = N, bufs: int = 3, repeats: int = 1):
    """Bass program for one core: out[r, :] = xs[r, :] * d[:] (d broadcast).

    xs: [ROWS_PER_CORE, N] f32, dr: [P, N] f32 (sign vector replicated on
    all 128 partitions), out: [ROWS_PER_CORE, N] f32.

    repeats > 1 re-runs the full streaming loop (same I/O, identical
    result) — used only to measure steady-state device time by slope.
    """
    import concourse.mybir as mybir
    import concourse.tile as tile
    from concourse import bacc

    f32 = mybir.dt.float32
    nc = bacc.Bacc("TRN2", target_bir_lowering=False, debug=False)

    xs = nc.dram_tensor("xs", [ROWS_PER_CORE, N], f32, kind="ExternalInput")
    dr = nc.dram_tensor("dr", [P, N], f32, kind="ExternalInput")
    out = nc.dram_tensor("out", [ROWS_PER_CORE, N], f32, kind="ExternalOutput")

    n_row_tiles = ROWS_PER_CORE // P
    n_col_tiles = N // free_cols

    with tile.TileContext(nc) as tc:
        with (
            tc.tile_pool(name="dpool", bufs=1) as dpool,
            tc.tile_pool(name="xpool", bufs=bufs) as xpool,
        ):
            dt_ = dpool.tile([P, N], f32)
            nc.sync.dma_start(dt_[:], dr[:, :])
            for _ in range(repeats):
                for i in range(n_row_tiles):
                    r0 = i * P
                    for j in range(n_col_tiles):
                        c0 = j * free_cols
                        xt = xpool.tile([P, free_cols], f32)
                        nc.sync.dma_start(
                            xt[:], xs[r0 : r0 + P, c0 : c0 + free_cols]
                        )
                        nc.vector.tensor_mul(xt[:], xt[:], dt_[:, c0 : c0 + free_cols])
                        nc.sync.dma_start(
                            out[r0 : r0 + P, c0 : c0 + free_cols], xt[:]
                        )
    nc.finalize()
    return nc


class _Exec:
    """Compile-once SPMD executor for a finalized Bass program.

    Mirrors concourse.bass2jax.run_bass_via_pjrt's multi-core branch, but
    traces/jits exactly once so repeat calls pay only transfer + exec.
    """

    def __init__(self, nc):
        import jax
        import concourse.mybir as mybir
        from concourse.bass2jax import (
            _bass_exec_p,
            install_neuronx_cc_hook,
            partition_id_tensor,
        )
        from jax.experimental.shard_map import shard_map
        from jax.sharding import Mesh, NamedSharding, PartitionSpec

        install_neuronx_cc_hook()
        self.jax = jax
        partition_name = (
            nc.partition_id_tensor.name if nc.partition_id_tensor else None
        )

        in_names, out_names, out_avals, zero_shapes = [], [], [], []
        for alloc in nc.m.functions[0].allocations:
            if not isinstance(alloc, mybir.MemoryLocationSet):
                continue
            name = alloc.memorylocations[0].name
            if alloc.kind == "ExternalInput":
                if name != partition_name:
                    in_names.append(name)
            elif alloc.kind == "ExternalOutput":
                out_names.append(name)
                shape = tuple(alloc.tensor_shape)
                dtype = mybir.dt.np(alloc.dtype)
                out_avals.append(jax.core.ShapedArray(shape, dtype))
                zero_shapes.append((shape, dtype))

        self.in_names = list(in_names)
        self.out_names = list(out_names)
        self.out_avals = out_avals
        n_params = len(in_names)
        n_outs = len(out_names)

        bind_in_names = in_names + out_names
        if partition_name is not None:
            bind_in_names.append(partition_name)

        def _body(*args):
            operands = list(args)
            if partition_name is not None:
                operands.append(partition_id_tensor())
            outs = _bass_exec_p.bind(
                *operands,
                out_avals=tuple(out_avals),
                in_names=tuple(bind_in_names),
                out_names=tuple(out_names),
                lowering_input_output_aliases=(),
                sim_require_finite=True,
                sim_require_nnan=True,
                nc=nc,
            )
            return tuple(outs)

        devices = jax.devices()[:N_CORES]
        assert len(devices) == N_CORES
        self.mesh = Mesh(np.asarray(devices), ("core",))
        pspec = PartitionSpec("core")
        in_specs = (pspec,) * (n_params + n_outs)
        out_specs = (pspec,) * n_outs
        donate = tuple(range(n_params, n_params + n_outs))
        self.sharding = NamedSharding(self.mesh, pspec)
        self.sharded = jax.jit(
            shard_map(
                _body,
                mesh=self.mesh,
                in_specs=in_specs,
                out_specs=out_specs,
                check_rep=False,
            ),
            donate_argnums=donate,
            keep_unused=True,
        )
        # on-device zero allocator (avoids shipping 256 MiB of zeros per call)
        self._zeros = jax.jit(
            lambda: tuple(
                jax.numpy.zeros((N_CORES * s[0], *s[1:]), dt)
                for (s, dt) in zero_shapes
            ),
            out_shardings=(self.sharding,) * n_outs,
        )

    def __call__(self, *concat_inputs):
        """concat_inputs: one array per in_name, core-shards concatenated on
        axis 0.  Returns list of np outputs (concat on axis 0)."""
        outs = self.sharded(*concat_inputs, *self._zeros())
        return outs


def _get_exec(repeats: int = 1) -> _Exec:
    key = ("exec", repeats)
    if key not in _CACHE:
        _CACHE[key] = _Exec(_build_nc(repeats=repeats))
    return _CACHE[key]


def _device_inputs(xs_flat: np.ndarray, d: np.ndarray):
    """Build the concatenated device inputs (xs concat over cores is just
    xs_flat itself; dr is the replicated sign tile, cached on device)."""
    import jax

    ex = _get_exec()
    key = ("dr_dev", d.tobytes())
    if key not in _CACHE:
        drep = np.ascontiguousarray(
            np.broadcast_to(d[None, :], (N_CORES * P, N)).astype(np.float32)
        )
        _CACHE[key] = jax.device_put(drep, ex.sharding)
    return _CACHE[key]


def _run_device(xs_flat: np.ndarray, d: np.ndarray) -> np.ndarray:
    ex = _get_exec()
    dr_dev = _device_inputs(xs_flat, d)
    (out,) = ex(xs_flat, dr_dev)
    return np.asarray(out)


def kernel(x: np.ndarray, tmat: np.ndarray) -> np.ndarray:
    x = np.asarray(x, dtype=np.float32)
    tmat = np.asarray(tmat, dtype=np.float32)
    assert x.shape == (B, C, N) and tmat.shape == (N, N)

    d = np.ascontiguousarray(np.diagonal(tmat))
    if not np.array_equal(tmat, np.diag(d)):
        # Non-diagonal transfer matrix: never happens for CPhaseLayer, but
        # keep a correct host fallback.
        return (x.reshape(ROWS, N).astype(np.float32) @ tmat).reshape(B, C, N)

    xs_flat = np.ascontiguousarray(x).reshape(ROWS, N)
    out = _run_device(xs_flat, d)
    return out.reshape(B, C, N).astype(np.float32)
